# revision 57
# baseline (speedup 1.0000x reference)
"""Tensor-parallel causal self-attention (MLA-style low-rank KV) for 8 trn2 cores.

Sharding: DP2 over batch x TP4 over heads. Core c -> batch b=c//4, head group
g=c%4 (8 heads each). Each core computes its batch's projections (q/k_rope/
low-rank c_kv shared down-proj), assembles+ropes k, runs causal attention for
its 8 heads, and produces a partial output (row-sharded Wo). Host sums the 4
TP partials per batch.

v2 pipeline:
- Projections from x run as fp8e4 hi/lo (error-compensated) DoubleRow matmuls:
  x and Wcat are pre-scaled and split host-side into hi + residual fp8, and
  x@W = xh@wh + xl@wh + xh@wl accumulates in PSUM (~1.3e-3 rel err, 0.75x
  bf16 cycles at 2 contraction-tiles per instruction).
- j-outer emission over 8-psum-bank groups so compute paces the x/w DMA
  stream (no big startup stall).
- Wuk is zero-padded host-side to 4 pair-aligned 128-col tiles and k_rope is
  folded into the same PSUM accumulation via a 0/1 placement-permutation
  matmul, so assembled k evacuates as one full-tile identity-partition copy
  (no scatter DMAs, no <32-partition-offset ops the BIR verifier rejects).
- RoPE rotation (the +-32-partition pair swap with sign) runs on PE as a
  signed permutation matmul; the rotation psum is staged to bf16 so the
  cos/sin muls + add run in DVE packed 2x mode.
- Attention S/AV in bf16 with exact causal width trimming; both heads of a
  pair share one 2-bank S psum tile so P=exp(S/8) is a single ACT op per key
  block; AV carries a ones-row to get softmax denominators for free.
- Emission interleaves independent PE work into every ACT/DVE-bound window:
  kc chains under the rope-evac trail, v(tn1) and qb1-S prefetch inside the
  qb0 rounds, Wo(tn0) chains at kb granularity inside qb1.
- Outputs are bf16 partials summed on host; DMAs are batched and spread
  across the SP/ACT HWDGE queues plus the gpsimd SWDGE path.
"""
import sys

sys.path.insert(0, "/opt/trn_rl_repo")

import numpy as np
import ml_dtypes

import concourse.bass as bass
import concourse.tile as tile
from concourse import bacc, mybir
from concourse.bass_utils import run_bass_kernel_spmd

F32 = mybir.dt.float32
BF16 = mybir.dt.bfloat16
FP8 = mybir.dt.float8e4

S, B, D = 1024, 2, 2048
TOK = S
TN = 2               # 512-token tiles
NJ = 8               # contraction kd-pairs (16 x 128 = 8 x 256)
NFM = 9              # q(4) + kr(1) + ckv(4)
LR = 512
NH = 8               # heads per core
DQK = 64
WS = 256.0           # weight pre-scale before fp8 split
XS = 4.0             # activation pre-scale
THETA = 10000.0

DR = mybir.MatmulPerfMode.DoubleRow


def build_program(debug_taps=False):
    nc = bacc.Bacc("TRN2", target_bir_lowering=False, debug=False)
    x_d = nc.dram_tensor("x", [128, NJ * 2 * 2 * TOK], FP8, kind="ExternalInput").ap()
    wc_d = nc.dram_tensor("wc", [128, NJ * 2 * NFM * 2 * 128], FP8, kind="ExternalInput").ap()
    wuk_d = nc.dram_tensor("wuk", [128, 4 * 4 * 128], BF16, kind="ExternalInput").ap()
    wuv_d = nc.dram_tensor("wuv", [128, 4 * 512], BF16, kind="ExternalInput").ap()
    wo_d = nc.dram_tensor("wo", [128, 16 * 4 * 128], BF16, kind="ExternalInput").ap()
    perm_d = nc.dram_tensor("perm", [128, 128], BF16, kind="ExternalInput").ap()
    pkr_d = nc.dram_tensor("pkr", [128, 512], BF16, kind="ExternalInput").ap()
    cosP_d = nc.dram_tensor("cosP", [128, TOK], BF16, kind="ExternalInput").ap()
    sinP_d = nc.dram_tensor("sinP", [128, TOK], BF16, kind="ExternalInput").ap()
    biask_d = nc.dram_tensor("biask", [128, 8], F32, kind="ExternalInput").ap()
    tri_d = nc.dram_tensor("tri", [128, 896], BF16, kind="ExternalInput").ap()
    outT_d = nc.dram_tensor("outT", [D, TOK], BF16, kind="ExternalOutput").ap()
    if debug_taps:
        dbg = {n: nc.dram_tensor(n, sh, BF16, kind="ExternalOutput").ap()
               for n, sh in [("dbg_qsb0", [128, 512]), ("dbg_ckv0", [128, 1024]),
                             ("dbg_kpre", [128, 4096]), ("dbg_qr0", [128, 1024]),
                             ("dbg_kr0", [128, 1024]), ("dbg_va0", [128, 520]),
                             ("dbg_at0", [128, 1024])]}

    DSC = 1.0 / (WS * XS)

    with tile.TileContext(nc) as tc:
        with tc.tile_pool(name="persist", bufs=1) as persist, \
             tc.tile_pool(name="small", bufs=4) as small, \
             tc.tile_pool(name="ps", bufs=4, space="PSUM") as ps, \
             tc.tile_pool(name="ps2", bufs=2, space="PSUM") as ps2:

            inx_ctx = tc.tile_pool(name="inx", bufs=1)
            inx = inx_ctx.__enter__()
            # ---- input DMAs (SP queue), interleaved x[j]/wc[j] for fast start
            xt = inx.tile([128, NJ * 2 * 2 * TOK], FP8, tag="x")
            wct = inx.tile([128, NJ * 2 * NFM * 2 * 128], FP8, tag="wc")
            XJ = 2 * 2 * TOK        # per-j x cols
            WJ = 2 * NFM * 2 * 128  # per-j wc cols
            def dma_cols(dst, dsrc, c0, c1):
                nc.sync.dma_start(out=dst[:, c0:c1], in_=dsrc[:, c0:c1])

            # first x chunk on the ACT HWDGE queue so its issue overlaps wc's
            nc.scalar.dma_start(out=xt[:, 0:XJ // 2], in_=x_d[:, 0:XJ // 2])
            for j in range(NJ):
                for h in range(2):
                    dma_cols(wct, wc_d, j * WJ + h * WJ // 2, j * WJ + (h + 1) * WJ // 2)
                    if j == 0 and h == 0:
                        continue
                    dma_cols(xt, x_d, j * XJ + h * XJ // 2, j * XJ + (h + 1) * XJ // 2)
            xv = xt[:].rearrange("p (j hl i n) -> p j hl i n", j=NJ, hl=2, i=2)
            wcv = wct[:].rearrange("p (j hl fm i m) -> p j hl fm i m",
                                   j=NJ, hl=2, fm=NFM, i=2)

            perm = persist.tile([128, 128], BF16, tag="perm")
            nc.sync.dma_start(out=perm, in_=perm_d)
            pkr = persist.tile([128, 512], BF16, tag="pkr")
            nc.sync.dma_start(out=pkr, in_=pkr_d)
            cosP = persist.tile([128, TOK], BF16, tag="cosP")
            nc.sync.dma_start(out=cosP, in_=cosP_d)
            sinP = persist.tile([128, TOK], BF16, tag="sinP")
            nc.sync.dma_start(out=sinP, in_=sinP_d)
            biask = persist.tile([128, 8], F32, tag="biask")
            nc.sync.dma_start(out=biask, in_=biask_d)
            tri = persist.tile([128, 896], BF16, tag="tri")
            nc.sync.dma_start(out=tri, in_=tri_d)
            wuk = persist.tile([128, 4 * 4 * 128], BF16, tag="wuk")
            nc.sync.dma_start(out=wuk, in_=wuk_d)
            wukv = wuk[:].rearrange("p (t kd m) -> p t kd m", t=4, kd=4)
            wuv = persist.tile([128, 4 * 512], BF16, tag="wuv")
            nc.sync.dma_start(out=wuv, in_=wuv_d)
            # ---- persistent activations
            qsb = [persist.tile([128, 512], BF16, name=f"qsb{i}", tag=f"qsb{i}")
                   for i in range(8)]                      # (qfm, tn)
            ckv = [persist.tile([128, TOK], BF16, name=f"ckv{k}", tag=f"ckv{k}")
                   for k in range(4)]
            kpre = persist.tile([128, 4 * TOK], BF16, tag="kpre")
            q_r = [persist.tile([128, TOK], BF16, name=f"qr{t}", tag=f"qr{t}")
                   for t in range(4)]
            k_r = [persist.tile([128, TOK], BF16, name=f"kr{t}", tag=f"kr{t}")
                   for t in range(4)]
            vaug = [persist.tile([128, NH * (DQK + 1)], BF16, name=f"va{k}", tag=f"va{k}")
                    for k in range(8)]
            attnT = [persist.tile([128, TOK], BF16, name=f"at{t}", tag=f"at{t}")
                     for t in range(4)]

            def proj_mms(ps_tile, fm, tn):
                """24 DoubleRow matmuls accumulating [Wcat.T x] tile (fm, tn)."""
                for j in range(NJ):
                    for ti, (whl, xhl) in enumerate([(0, 0), (0, 1), (1, 0)]):
                        nc.tensor.matmul(
                            out=ps_tile[:],
                            lhsT=wcv[:, j, whl, fm],
                            rhs=xv[:, j, xhl, :, tn * 512:(tn + 1) * 512],
                            start=(j == 0 and ti == 0),
                            stop=(j == NJ - 1 and ti == 2),
                            perf_mode=DR,
                        )

            # ---- projection groups A/B: j-outer over 8 psum banks
            for slots in ([(0, 0), (0, 1), (1, 0), (1, 1),
                           (5, 0), (5, 1), (6, 0), (6, 1)],
                          [(2, 0), (2, 1), (3, 0), (3, 1),
                           (7, 0), (7, 1), (8, 0), (8, 1)]):
                pst = {}
                for si in range(0, 4):
                    pst[slots[si]] = ps.tile([128, 512], F32, name="pp", tag="ps")
                for si in range(4, 8, 2):
                    dbl = ps2.tile([128, 1024], F32, name="pp2", tag="s2")
                    pst[slots[si]] = dbl[:, 0:512]
                    pst[slots[si + 1]] = dbl[:, 512:1024]
                for j in range(NJ):
                    for s in slots:
                        fm, tn = s
                        for ti, (whl, xhl) in enumerate([(0, 0), (0, 1), (1, 0)]):
                            nc.tensor.matmul(
                                out=pst[s][:],
                                lhsT=wcv[:, j, whl, fm],
                                rhs=xv[:, j, xhl, :, tn * 512:(tn + 1) * 512],
                                start=(j == 0 and ti == 0),
                                stop=(j == NJ - 1 and ti == 2),
                                perf_mode=DR,
                            )
                for s in slots:
                    fm, tn = s
                    if fm < 4:      # q -> descale to bf16 staging
                        nc.scalar.mul(qsb[fm * 2 + tn][:], pst[s][:], DSC)
                    else:           # ckv (fm 5..8) -> descale to bf16
                        nc.scalar.mul(ckv[fm - 5][:, tn * 512:(tn + 1) * 512],
                                      pst[s][:], DSC)

            # ---- kr (both tn), staged to bf16 for the kc-psum fold
            kprev = kpre[:].rearrange("p (t n) -> p t n", t=4)
            krsbs = []
            for tn in range(TN):
                kr_ps = ps.tile([128, 512], F32, name="krp", tag="ps")
                proj_mms(kr_ps, 4, tn)
                krsb = small.tile([128, 512], BF16, tag="krsb", bufs=2)
                nc.scalar.mul(krsb[:], kr_ps[:], DSC)
                krsbs.append(krsb)

            inx_ctx.__exit__(None, None, None)
            late_ctx = tc.tile_pool(name="late", bufs=1)
            late = late_ctx.__enter__()
            wot = late.tile([128, 16 * 4 * 128], BF16, tag="wo")
            for h in range(2):
                nc.sync.dma_start(out=wot[:, h * 4096:(h + 1) * 4096],
                                  in_=wo_d[:, h * 4096:(h + 1) * 4096])
            wov = wot[:].rearrange("p (dm kd m) -> p dm kd m", dm=16, kd=4)
            late_small = late_ctx2 = tc.tile_pool(name="lsmall", bufs=4)
            late_small = late_ctx2.__enter__()

            # ---- per-tn: q-rope, kc + k-rope, v
            def rope(dst, src_sb, tn, tag):
                """dst[:, tn*512:+512] = src*cos + (perm.T@src)*sin.

                The rotation psum is evacuated to bf16 so every DVE op is
                all-bf16/SBUF and runs in the 2x/4x packed modes."""
                cols = slice(tn * 512, (tn + 1) * 512)
                rot = ps.tile([128, 512], F32, tag="ps", name="rot")
                nc.tensor.matmul(out=rot[:], lhsT=perm[:], rhs=src_sb,
                                 start=True, stop=True)
                rotsb = small.tile([128, 512], BF16, tag="rotsb", bufs=3)
                nc.scalar.copy(out=rotsb, in_=rot[:])
                t1 = small.tile([128, 512], BF16, tag="t1", bufs=2)
                nc.vector.tensor_mul(t1, src_sb, cosP[:, cols])
                t2 = small.tile([128, 512], BF16, tag="t2", bufs=2)
                nc.vector.tensor_mul(t2, rotsb, sinP[:, cols])
                nc.vector.tensor_add(dst[:, cols], t1, t2)

            def kc_chain(t, tn, evac_eng):
                """k assembly: 4 Wuk tiles (nope dims, rope rows zero-padded)
                + one 0/1-permutation matmul folding k_rope into its rows."""
                pk = ps.tile([128, 512], F32, tag="ps", name="pk")
                for kd in range(4):
                    nc.tensor.matmul(
                        out=pk[:],
                        lhsT=wukv[:, t, kd],
                        rhs=ckv[kd][:, tn * 512:(tn + 1) * 512],
                        start=(kd == 0), stop=False,
                    )
                nc.tensor.matmul(
                    out=pk[:], lhsT=pkr[:, t * 128:(t + 1) * 128],
                    rhs=krsbs[tn][:], start=False, stop=True,
                )
                evac_eng(out=kprev[:, t, tn * 512:(tn + 1) * 512], in_=pk[:])

            # ones rows of all vaug tiles set once, off the critical path
            for kt in range(8):
                nc.gpsimd.memset(vaug[kt][:], 1.0)

            def v_chain(kt, evac=None):
                pv = ps.tile([128, 512], F32, tag="ps", name="pv")
                for kd in range(4):
                    nc.tensor.matmul(
                        out=pv[:],
                        lhsT=ckv[kd][:, kt * 128:(kt + 1) * 128],
                        rhs=wuv[:, kd * 512:(kd + 1) * 512],
                        start=(kd == 0), stop=(kd == 3),
                    )
                (evac or nc.scalar.copy)(
                    out=vaug[kt][:].rearrange("p (h dd) -> p h dd", h=NH)[:, :, 0:DQK],
                    in_=pv[:].rearrange("p (h d) -> p h d", h=NH),
                )

            # production: qrot/kc interleaved (kc's 5-matmul chains cover the
            # rot evac latency); v last so attention psum allocations chase
            # fast ACT pv-evacs, not the trailing DVE rope chains
            for tn in range(TN):
                for t in range(4):
                    rope(q_r[t], qsb[t * 2 + tn][:], tn, "q")
                    kc_chain(t, tn, nc.scalar.copy if t % 2 == 0 else
                             nc.vector.tensor_copy)
            for tn in range(TN):
                for t in range(4):
                    rope(k_r[t], kprev[:, t, tn * 512:(tn + 1) * 512], tn, "k")
                    if tn == 0 and t >= 2:
                        v_chain(t - 2)
            for kt in range(2, 4):
                v_chain(kt)

            if debug_taps:
                nc.sync.dma_start(out=dbg["dbg_qsb0"], in_=qsb[0][:])
                nc.sync.dma_start(out=dbg["dbg_ckv0"], in_=ckv[0][:])
                nc.sync.dma_start(out=dbg["dbg_kpre"], in_=kpre[:, 0:4096])
                nc.sync.dma_start(out=dbg["dbg_qr0"], in_=q_r[0][:])
                nc.sync.dma_start(out=dbg["dbg_kr0"], in_=k_r[0][:])
                nc.sync.dma_start(out=dbg["dbg_va0"], in_=vaug[0][:])

            # ---- attention qb, with wo(tn0) interleaved into qb1
            def attn_S_kb(t, qb, kb):
                """S + exp + mask for one key block: both heads' S blocks live
                in a single 2-bank psum tile so exp is one ACT op per kb."""
                d = kb * 128 - qb * 512
                sd = max(d, 0)
                s2 = ps2.tile([128, 1024], F32, tag="s2", name="s2")
                for hh in range(2):
                    nc.tensor.matmul(
                        out=s2[:, hh * 512 + sd:hh * 512 + 512],
                        lhsT=k_r[t][64 * hh:64 * hh + 64, kb * 128:(kb + 1) * 128],
                        rhs=q_r[t][64 * hh:64 * hh + 64,
                                   qb * 512 + sd:(qb + 1) * 512],
                        start=True, stop=True,
                    )
                p_t = late_small.tile([128, 1024], BF16, tag="p", bufs=28)
                nc.scalar.activation(
                    p_t[:].rearrange("p (g n) -> p g n", g=2)[:, :, sd:512],
                    s2[:].rearrange("p (g n) -> p g n", g=2)[:, :, sd:512],
                    mybir.ActivationFunctionType.Exp,
                    bias=biask[:, kb:kb + 1], scale=0.125)
                if kb >= 4 * qb:    # diagonal block: causal mask
                    s0 = 384 - d
                    for hh in range(2):
                        meng = nc.vector
                        meng.tensor_mul(
                            p_t[:, hh * 512 + sd:hh * 512 + 512],
                            p_t[:, hh * 512 + sd:hh * 512 + 512],
                            tri[:, s0 + sd:s0 + 512])
                return p_t

            def attn_S(t, qb):
                return [attn_S_kb(t, qb, kb) for kb in range(4 * qb + 4)]

            def attn_AV(t, qb, p_ts):
                nkb = 4 * qb + 4
                av = [ps.tile([65, 512], F32, name="av", tag="ps") for _ in range(2)]
                for kb in range(nkb):
                    d = kb * 128 - qb * 512
                    sd = max(d, 0)
                    for hh in range(2):
                        h = 2 * t + hh
                        nc.tensor.matmul(
                            out=av[hh][:, sd:512],
                            lhsT=vaug[kb][:, h * 65:h * 65 + 65],
                            rhs=p_ts[kb][:, hh * 512 + sd:hh * 512 + 512],
                            start=(kb == 0), stop=(kb == nkb - 1),
                        )
                for hh in range(2):
                    rec = small.tile([1, 512], F32, tag="rec", bufs=2)
                    nc.vector.reciprocal(rec, av[hh][64:65, :])
                    bc = small.tile([64, 512], F32, tag="bc", bufs=2)
                    nc.gpsimd.partition_broadcast(bc[:], rec[:], channels=64)
                    nc.vector.tensor_mul(
                        attnT[t][64 * hh:64 * hh + 64, qb * 512:(qb + 1) * 512],
                        av[hh][0:64, :], bc)

            def wo_chain(dm, tn, qi):
                po = ps.tile([128, 512], F32, tag="ps", name="po")
                for kd in range(4):
                    nc.tensor.matmul(
                        out=po[:],
                        lhsT=wov[:, dm, kd],
                        rhs=attnT[kd][:, tn * 512:(tn + 1) * 512],
                        start=(kd == 0), stop=(kd == 3),
                    )
                osb = late_small.tile([128, 512], BF16, tag="osb", bufs=12)
                if tn == 0:
                    nc.vector.tensor_copy(out=osb, in_=po[:])
                    eng = (nc.sync, nc.gpsimd)[qi % 2]
                else:
                    if qi % 2 == 0:
                        nc.scalar.copy(out=osb, in_=po[:])
                    else:
                        nc.vector.tensor_copy(out=osb, in_=po[:])
                    eng = (nc.sync, nc.gpsimd)[qi % 2]
                eng.dma_start(
                    out=outT_d[dm * 128:(dm + 1) * 128, tn * 512:(tn + 1) * 512],
                    in_=osb)

            # qb0: stagger S/AV across t to hide exp latency
            p_q0 = {}
            p_q0[0] = attn_S(0, 0)
            v_chain(4)
            p_q0[1] = attn_S(1, 0)
            v_chain(5)
            p_q0[2] = attn_S(2, 0)
            attn_AV(0, 0, p_q0[0])
            v_chain(6)
            p_q0[3] = attn_S(3, 0)
            attn_AV(1, 0, p_q0[1])
            v_chain(7)
            attn_AV(2, 0, p_q0[2])
            # prefetch qb1 t=0 S pieces around the qb0 tail AVs
            p_q1 = [attn_S_kb(0, 1, 0), attn_S_kb(0, 1, 1), attn_S_kb(0, 1, 2),
                    attn_S_kb(0, 1, 3)]
            attn_AV(3, 0, p_q0[3])

            if debug_taps:
                nc.sync.dma_start(out=dbg["dbg_at0"], in_=attnT[0][:])
            # qb1: wo(tn0) chains interleaved at kb granularity so PE has
            # independent work while ACT catches up on exps
            wo_q = list(range(16))
            qi = 0
            for t in range(4):
                p_ts = p_q1 if t == 0 else []
                for kb in (range(4, 8) if t == 0 else range(8)):
                    p_ts.append(attn_S_kb(t, 1, kb))
                    if kb % 2 == 1 and wo_q:
                        wo_chain(wo_q.pop(0), 0, qi)
                        qi += 1
                attn_AV(t, 1, p_ts)
            while wo_q:
                wo_chain(wo_q.pop(0), 0, qi)
                qi += 1
            for dm in range(16):
                wo_chain(dm, 1, qi)
                qi += 1
            late_ctx2.__exit__(None, None, None)
            late_ctx.__exit__(None, None, None)

    nc.compile()
    return nc


_CACHE = {}


def _get_program():
    if "nc" not in _CACHE:
        _CACHE["nc"] = build_program()
    return _CACHE["nc"]


def _fp8_split(a):
    f8 = ml_dtypes.float8_e4m3
    hi = a.astype(f8)
    lo = (a - hi.astype(np.float32)).astype(f8)
    return hi, lo


def make_in_maps(hidden_states, sequence_mask, Wq, Wkr, Wdk, Wuk, Wuv, Wo):
    hidden_states = np.asarray(hidden_states, dtype=np.float32)
    sequence_mask = np.asarray(sequence_mask).astype(bool)
    Wq, Wkr, Wdk = (np.asarray(a, np.float32) for a in (Wq, Wkr, Wdk))
    Wuk, Wuv, Wo = (np.asarray(a, np.float32) for a in (Wuk, Wuv, Wo))
    bf = ml_dtypes.bfloat16

    inv_freq = (1.0 / (THETA ** (np.arange(0, 32, dtype=np.float32) / 32.0))).astype(np.float32)
    tri = (np.arange(896)[None, :] >= (np.arange(128)[:, None] + 384)).astype(bf)
    # signed rope rotation permutation: dst d<32 <- -src[d+32]; d>=32 <- +src[d-32]
    perm = np.zeros((128, 128), np.float32)
    for hh in range(2):
        for dd in range(32):
            perm[hh * 64 + dd + 32, hh * 64 + dd] = -1.0
            perm[hh * 64 + dd, hh * 64 + dd + 32] = 1.0
    perm = perm.astype(bf)
    pkr = np.zeros((128, 512), np.float32)
    for t in range(4):
        for e in range(2):
            for p in range(16):
                pkr[t * 32 + e * 16 + p, t * 128 + e * 64 + p] = 1.0
    pkr = pkr.astype(bf)

    per_g = []
    for g in range(4):
        wcat = np.concatenate(
            [Wq[:, g * 512:(g + 1) * 512], Wkr[:, g * 128:(g + 1) * 128], Wdk],
            axis=1) * WS  # [2048, 1152]
        wh, wl = _fp8_split(wcat)
        # [2048=(j,i,p), 1152=(fm,m)] -> [128p, j, hl, fm, i, m]
        def packw(a):
            return a.reshape(NJ, 2, 128, NFM, 128).transpose(2, 0, 3, 1, 4)
        wc = np.stack([packw(wh), packw(wl)], axis=2)  # [128, j, hl, fm, i, m]
        wc = np.ascontiguousarray(wc.reshape(128, -1))

        wuk_g = Wuk[:, g * 384:(g + 1) * 384]
        wuk_pad = np.zeros((LR, 4 * 128), np.float32)
        for t in range(4):
            wuk_pad[:, t * 128 + 16:t * 128 + 64] = wuk_g[:, (2 * t) * 48:(2 * t + 1) * 48]
            wuk_pad[:, t * 128 + 80:t * 128 + 128] = wuk_g[:, (2 * t + 1) * 48:(2 * t + 2) * 48]
        # [512=(kd,p), 512=(t,m)] -> [128p, t, kd, m]
        wuk_sb = np.ascontiguousarray(
            wuk_pad.reshape(4, 128, 4, 128).transpose(1, 2, 0, 3).reshape(128, -1)).astype(bf)
        wuv_sb = np.ascontiguousarray(
            Wuv[:, g * 512:(g + 1) * 512].reshape(4, 128, 512).transpose(1, 0, 2)
            .reshape(128, -1)).astype(bf)
        wo_g = Wo[g * 512:(g + 1) * 512, :]  # [512=(kd,p), 2048=(dm,m)]
        wo_sb = np.ascontiguousarray(
            wo_g.reshape(4, 128, 16, 128).transpose(1, 2, 0, 3).reshape(128, -1)).astype(bf)
        per_g.append((wc, wuk_sb, wuv_sb, wo_sb))

    per_b = []
    for b in range(B):
        xs = hidden_states[:, b, :].T * XS  # [2048, 1024]
        xh, xl = _fp8_split(xs)
        # [2048=(j,i,p), 1024] -> [128p, j, hl, i, n]
        def packx(a):
            return a.reshape(NJ, 2, 128, TOK).transpose(2, 0, 1, 3)
        xp = np.stack([packx(xh), packx(xl)], axis=2)  # [128, j, hl, i, n]
        xp = np.ascontiguousarray(xp.reshape(128, -1))
        pos = np.cumsum(sequence_mask[b].astype(np.int32)) - 1
        ang = pos.astype(np.float32)[None, :] * inv_freq[:, None]  # [32, 1024]
        cosP = np.ascontiguousarray(np.tile(np.cos(ang), (4, 1))).astype(np.float32)
        sinP = np.ascontiguousarray(np.tile(np.sin(ang), (4, 1))).astype(np.float32)
        biask = np.ascontiguousarray(
            np.where(sequence_mask[b], 0.0, -30.0).astype(np.float32).reshape(8, 128).T)
        per_b.append((xp, cosP.astype(bf), sinP.astype(bf), biask))

    in_maps = []
    for c in range(8):
        b, g = c // 4, c % 4
        wc, wuk_sb, wuv_sb, wo_sb = per_g[g]
        xp, cosP, sinP, biask = per_b[b]
        in_maps.append({
            "x": xp, "wc": wc, "wuk": wuk_sb, "wuv": wuv_sb, "wo": wo_sb,
            "perm": perm, "pkr": pkr, "cosP": cosP, "sinP": sinP,
            "biask": biask, "tri": tri,
        })
    return in_maps


def kernel(hidden_states, sequence_mask, Wq, Wkr, Wdk, Wuk, Wuv, Wo, _trace=False):
    nc = _get_program()
    in_maps = make_in_maps(hidden_states, sequence_mask, Wq, Wkr, Wdk, Wuk, Wuv, Wo)
    if _trace:
        try:
            res = run_bass_kernel_spmd(nc, in_maps, core_ids=list(range(8)), trace=True)
        except Exception:
            res = run_bass_kernel_spmd(nc, in_maps, core_ids=list(range(8)))
    else:
        res = run_bass_kernel_spmd(nc, in_maps, core_ids=list(range(8)))
    mask = np.asarray(sequence_mask).astype(np.float32)  # [B, S]
    out = np.empty((B, S, D), dtype=np.float32)
    for b in range(B):
        acc = np.zeros((D, TOK), dtype=np.float64)
        for g in range(4):
            acc += res.results[4 * b + g]["outT"].astype(np.float64)
        out[b] = acc.T.astype(np.float32) * mask[b][:, None]
    if _trace:
        kernel._last_results = res
    return out


# revision 59
# speedup vs baseline: 1.0007x; 1.0007x over previous
"""Tensor-parallel causal self-attention (MLA-style low-rank KV) for 8 trn2 cores.

Sharding: DP2 over batch x TP4 over heads. Core c -> batch b=c//4, head group
g=c%4 (8 heads each). Each core computes its batch's projections (q/k_rope/
low-rank c_kv shared down-proj), assembles+ropes k, runs causal attention for
its 8 heads, and produces a partial output (row-sharded Wo). Host sums the 4
TP partials per batch.

v2 pipeline:
- Projections from x run as fp8e4 hi/lo (error-compensated) DoubleRow matmuls:
  x and Wcat are pre-scaled and split host-side into hi + residual fp8, and
  x@W = xh@wh + xl@wh + xh@wl accumulates in PSUM (~1.3e-3 rel err, 0.75x
  bf16 cycles at 2 contraction-tiles per instruction).
- j-outer emission over 8-psum-bank groups so compute paces the x/w DMA
  stream (no big startup stall).
- Wuk is zero-padded host-side to 4 pair-aligned 128-col tiles and k_rope is
  folded into the same PSUM accumulation via a 0/1 placement-permutation
  matmul, so assembled k evacuates as one full-tile identity-partition copy
  (no scatter DMAs, no <32-partition-offset ops the BIR verifier rejects).
- RoPE rotation (the +-32-partition pair swap with sign) runs on PE as a
  signed permutation matmul; the rotation psum is staged to bf16 so the
  cos/sin muls + add run in DVE packed 2x mode.
- Attention S/AV in bf16 with exact causal width trimming; both heads of a
  pair share one 2-bank S psum tile so P=exp(S/8) is a single ACT op per key
  block; AV carries a ones-row to get softmax denominators for free.
- Emission interleaves independent PE work into every ACT/DVE-bound window:
  kc chains under the rope-evac trail, v(tn1) and qb1-S prefetch inside the
  qb0 rounds, Wo(tn0) chains at kb granularity inside qb1.
- Outputs are bf16 partials summed on host; DMAs are batched and spread
  across the SP/ACT HWDGE queues plus the gpsimd SWDGE path.
"""
import sys

sys.path.insert(0, "/opt/trn_rl_repo")

import numpy as np
import ml_dtypes

import concourse.bass as bass
import concourse.tile as tile
from concourse import bacc, mybir
from concourse.bass_utils import run_bass_kernel_spmd

F32 = mybir.dt.float32
BF16 = mybir.dt.bfloat16
FP8 = mybir.dt.float8e4

S, B, D = 1024, 2, 2048
TOK = S
TN = 2               # 512-token tiles
NJ = 8               # contraction kd-pairs (16 x 128 = 8 x 256)
NFM = 9              # q(4) + kr(1) + ckv(4)
LR = 512
NH = 8               # heads per core
DQK = 64
WS = 256.0           # weight pre-scale before fp8 split
XS = 4.0             # activation pre-scale
THETA = 10000.0

DR = mybir.MatmulPerfMode.DoubleRow


def build_program(debug_taps=False):
    nc = bacc.Bacc("TRN2", target_bir_lowering=False, debug=False)
    x_d = nc.dram_tensor("x", [128, NJ * 2 * 2 * TOK], FP8, kind="ExternalInput").ap()
    wc_d = nc.dram_tensor("wc", [128, NJ * 2 * NFM * 2 * 128], FP8, kind="ExternalInput").ap()
    wuk_d = nc.dram_tensor("wuk", [128, 4 * 4 * 128], BF16, kind="ExternalInput").ap()
    wuv_d = nc.dram_tensor("wuv", [128, 4 * 512], BF16, kind="ExternalInput").ap()
    wo_d = nc.dram_tensor("wo", [128, 16 * 4 * 128], BF16, kind="ExternalInput").ap()
    perm_d = nc.dram_tensor("perm", [128, 128], BF16, kind="ExternalInput").ap()
    pkr_d = nc.dram_tensor("pkr", [128, 512], BF16, kind="ExternalInput").ap()
    cosP_d = nc.dram_tensor("cosP", [128, TOK], BF16, kind="ExternalInput").ap()
    sinP_d = nc.dram_tensor("sinP", [128, TOK], BF16, kind="ExternalInput").ap()
    biask_d = nc.dram_tensor("biask", [128, 8], F32, kind="ExternalInput").ap()
    tri_d = nc.dram_tensor("tri", [128, 896], BF16, kind="ExternalInput").ap()
    outT_d = nc.dram_tensor("outT", [D, TOK], BF16, kind="ExternalOutput").ap()
    if debug_taps:
        dbg = {n: nc.dram_tensor(n, sh, BF16, kind="ExternalOutput").ap()
               for n, sh in [("dbg_qsb0", [128, 512]), ("dbg_ckv0", [128, 1024]),
                             ("dbg_kpre", [128, 4096]), ("dbg_qr0", [128, 1024]),
                             ("dbg_kr0", [128, 1024]), ("dbg_va0", [128, 520]),
                             ("dbg_at0", [128, 1024])]}

    DSC = 1.0 / (WS * XS)

    with tile.TileContext(nc) as tc:
        with tc.tile_pool(name="persist", bufs=1) as persist, \
             tc.tile_pool(name="small", bufs=4) as small, \
             tc.tile_pool(name="ps", bufs=4, space="PSUM") as ps, \
             tc.tile_pool(name="ps2", bufs=2, space="PSUM") as ps2:

            inx_ctx = tc.tile_pool(name="inx", bufs=1)
            inx = inx_ctx.__enter__()
            # ---- input DMAs (SP queue), interleaved x[j]/wc[j] for fast start
            xt = inx.tile([128, NJ * 2 * 2 * TOK], FP8, tag="x")
            wct = inx.tile([128, NJ * 2 * NFM * 2 * 128], FP8, tag="wc")
            XJ = 2 * 2 * TOK        # per-j x cols
            WJ = 2 * NFM * 2 * 128  # per-j wc cols
            def dma_cols(dst, dsrc, c0, c1):
                nc.sync.dma_start(out=dst[:, c0:c1], in_=dsrc[:, c0:c1])

            # first x chunk on the ACT HWDGE queue so its issue overlaps wc's
            nc.scalar.dma_start(out=xt[:, 0:XJ // 2], in_=x_d[:, 0:XJ // 2])
            for j in range(NJ):
                for h in range(2):
                    dma_cols(wct, wc_d, j * WJ + h * WJ // 2, j * WJ + (h + 1) * WJ // 2)
                    if j == 0 and h == 0:
                        continue
                    dma_cols(xt, x_d, j * XJ + h * XJ // 2, j * XJ + (h + 1) * XJ // 2)
            xv = xt[:].rearrange("p (j hl i n) -> p j hl i n", j=NJ, hl=2, i=2)
            wcv = wct[:].rearrange("p (j hl fm i m) -> p j hl fm i m",
                                   j=NJ, hl=2, fm=NFM, i=2)

            perm = persist.tile([128, 128], BF16, tag="perm")
            nc.sync.dma_start(out=perm, in_=perm_d)
            pkr = persist.tile([128, 512], BF16, tag="pkr")
            nc.sync.dma_start(out=pkr, in_=pkr_d)
            cosP = persist.tile([128, TOK], BF16, tag="cosP")
            nc.sync.dma_start(out=cosP, in_=cosP_d)
            sinP = persist.tile([128, TOK], BF16, tag="sinP")
            nc.sync.dma_start(out=sinP, in_=sinP_d)
            biask = persist.tile([128, 8], F32, tag="biask")
            nc.sync.dma_start(out=biask, in_=biask_d)
            tri = persist.tile([128, 896], BF16, tag="tri")
            nc.sync.dma_start(out=tri, in_=tri_d)
            wuk = persist.tile([128, 4 * 4 * 128], BF16, tag="wuk")
            nc.sync.dma_start(out=wuk, in_=wuk_d)
            wukv = wuk[:].rearrange("p (t kd m) -> p t kd m", t=4, kd=4)
            wuv = persist.tile([128, 4 * 512], BF16, tag="wuv")
            nc.sync.dma_start(out=wuv, in_=wuv_d)
            # ---- persistent activations
            qsb = [persist.tile([128, 512], BF16, name=f"qsb{i}", tag=f"qsb{i}")
                   for i in range(8)]                      # (qfm, tn)
            ckv = [persist.tile([128, TOK], BF16, name=f"ckv{k}", tag=f"ckv{k}")
                   for k in range(4)]
            kpre = persist.tile([128, 4 * TOK], BF16, tag="kpre")
            q_r = [persist.tile([128, TOK], BF16, name=f"qr{t}", tag=f"qr{t}")
                   for t in range(4)]
            k_r = [persist.tile([128, TOK], BF16, name=f"kr{t}", tag=f"kr{t}")
                   for t in range(4)]
            vaug = [persist.tile([128, NH * (DQK + 1)], BF16, name=f"va{k}", tag=f"va{k}")
                    for k in range(8)]
            attnT = [persist.tile([128, TOK], BF16, name=f"at{t}", tag=f"at{t}")
                     for t in range(4)]

            def proj_mms(ps_tile, fm, tn):
                """24 DoubleRow matmuls accumulating [Wcat.T x] tile (fm, tn)."""
                for j in range(NJ):
                    for ti, (whl, xhl) in enumerate([(0, 0), (0, 1), (1, 0)]):
                        nc.tensor.matmul(
                            out=ps_tile[:],
                            lhsT=wcv[:, j, whl, fm],
                            rhs=xv[:, j, xhl, :, tn * 512:(tn + 1) * 512],
                            start=(j == 0 and ti == 0),
                            stop=(j == NJ - 1 and ti == 2),
                            perf_mode=DR,
                        )

            # ---- projection groups A/B: j-outer over 8 psum banks
            for slots in ([(0, 0), (0, 1), (1, 0), (1, 1),
                           (5, 0), (5, 1), (6, 0), (6, 1)],
                          [(2, 0), (2, 1), (3, 0), (3, 1),
                           (7, 0), (7, 1), (8, 0), (8, 1)]):
                pst = {}
                for si in range(0, 4):
                    pst[slots[si]] = ps.tile([128, 512], F32, name="pp", tag="ps")
                for si in range(4, 8, 2):
                    dbl = ps2.tile([128, 1024], F32, name="pp2", tag="s2")
                    pst[slots[si]] = dbl[:, 0:512]
                    pst[slots[si + 1]] = dbl[:, 512:1024]
                for j in range(NJ):
                    for s in slots:
                        fm, tn = s
                        for ti, (whl, xhl) in enumerate([(0, 0), (0, 1), (1, 0)]):
                            nc.tensor.matmul(
                                out=pst[s][:],
                                lhsT=wcv[:, j, whl, fm],
                                rhs=xv[:, j, xhl, :, tn * 512:(tn + 1) * 512],
                                start=(j == 0 and ti == 0),
                                stop=(j == NJ - 1 and ti == 2),
                                perf_mode=DR,
                            )
                for s in slots:
                    fm, tn = s
                    if fm < 4:      # q -> descale to bf16 staging
                        nc.scalar.mul(qsb[fm * 2 + tn][:], pst[s][:], DSC)
                    else:           # ckv (fm 5..8) -> descale to bf16
                        nc.scalar.mul(ckv[fm - 5][:, tn * 512:(tn + 1) * 512],
                                      pst[s][:], DSC)

            # ---- kr (both tn), staged to bf16 for the kc-psum fold
            kprev = kpre[:].rearrange("p (t n) -> p t n", t=4)
            krsbs = []
            for tn in range(TN):
                kr_ps = ps.tile([128, 512], F32, name="krp", tag="ps")
                proj_mms(kr_ps, 4, tn)
                krsb = small.tile([128, 512], BF16, tag="krsb", bufs=3)
                nc.scalar.mul(krsb[:], kr_ps[:], DSC)
                krsbs.append(krsb)

            inx_ctx.__exit__(None, None, None)
            late_ctx = tc.tile_pool(name="late", bufs=1)
            late = late_ctx.__enter__()
            wot = late.tile([128, 16 * 4 * 128], BF16, tag="wo")
            for h in range(2):
                nc.sync.dma_start(out=wot[:, h * 4096:(h + 1) * 4096],
                                  in_=wo_d[:, h * 4096:(h + 1) * 4096])
            wov = wot[:].rearrange("p (dm kd m) -> p dm kd m", dm=16, kd=4)
            late_small = late_ctx2 = tc.tile_pool(name="lsmall", bufs=4)
            late_small = late_ctx2.__enter__()

            # ---- per-tn: q-rope, kc + k-rope, v
            def rope(dst, src_sb, tn, tag):
                """dst[:, tn*512:+512] = src*cos + (perm.T@src)*sin.

                The rotation psum is evacuated to bf16 so every DVE op is
                all-bf16/SBUF and runs in the 2x/4x packed modes."""
                cols = slice(tn * 512, (tn + 1) * 512)
                rot = ps.tile([128, 512], F32, tag="ps", name="rot")
                nc.tensor.matmul(out=rot[:], lhsT=perm[:], rhs=src_sb,
                                 start=True, stop=True)
                rotsb = small.tile([128, 512], BF16, tag="rotsb", bufs=4)
                nc.scalar.copy(out=rotsb, in_=rot[:])
                t1 = small.tile([128, 512], BF16, tag="t1", bufs=3)
                nc.vector.tensor_mul(t1, src_sb, cosP[:, cols])
                t2 = small.tile([128, 512], BF16, tag="t2", bufs=3)
                nc.vector.tensor_mul(t2, rotsb, sinP[:, cols])
                nc.vector.tensor_add(dst[:, cols], t1, t2)

            def kc_chain(t, tn, evac_eng):
                """k assembly: 4 Wuk tiles (nope dims, rope rows zero-padded)
                + one 0/1-permutation matmul folding k_rope into its rows."""
                pk = ps.tile([128, 512], F32, tag="ps", name="pk")
                for kd in range(4):
                    nc.tensor.matmul(
                        out=pk[:],
                        lhsT=wukv[:, t, kd],
                        rhs=ckv[kd][:, tn * 512:(tn + 1) * 512],
                        start=(kd == 0), stop=False,
                    )
                nc.tensor.matmul(
                    out=pk[:], lhsT=pkr[:, t * 128:(t + 1) * 128],
                    rhs=krsbs[tn][:], start=False, stop=True,
                )
                evac_eng(out=kprev[:, t, tn * 512:(tn + 1) * 512], in_=pk[:])

            # ones rows of all vaug tiles set once, off the critical path
            for kt in range(8):
                nc.gpsimd.memset(vaug[kt][:], 1.0)

            def v_chain(kt, evac=None):
                pv = ps.tile([128, 512], F32, tag="ps", name="pv")
                for kd in range(4):
                    nc.tensor.matmul(
                        out=pv[:],
                        lhsT=ckv[kd][:, kt * 128:(kt + 1) * 128],
                        rhs=wuv[:, kd * 512:(kd + 1) * 512],
                        start=(kd == 0), stop=(kd == 3),
                    )
                (evac or nc.scalar.copy)(
                    out=vaug[kt][:].rearrange("p (h dd) -> p h dd", h=NH)[:, :, 0:DQK],
                    in_=pv[:].rearrange("p (h d) -> p h d", h=NH),
                )

            # production: qrot/kc interleaved (kc's 5-matmul chains cover the
            # rot evac latency); v last so attention psum allocations chase
            # fast ACT pv-evacs, not the trailing DVE rope chains
            for tn in range(TN):
                for t in range(4):
                    rope(q_r[t], qsb[t * 2 + tn][:], tn, "q")
                    kc_chain(t, tn, nc.scalar.copy if t % 2 == 0 else
                             nc.vector.tensor_copy)
            for tn in range(TN):
                for t in range(4):
                    rope(k_r[t], kprev[:, t, tn * 512:(tn + 1) * 512], tn, "k")
                    if tn == 0 and t >= 2:
                        v_chain(t - 2)
            for kt in range(2, 4):
                v_chain(kt)

            if debug_taps:
                nc.sync.dma_start(out=dbg["dbg_qsb0"], in_=qsb[0][:])
                nc.sync.dma_start(out=dbg["dbg_ckv0"], in_=ckv[0][:])
                nc.sync.dma_start(out=dbg["dbg_kpre"], in_=kpre[:, 0:4096])
                nc.sync.dma_start(out=dbg["dbg_qr0"], in_=q_r[0][:])
                nc.sync.dma_start(out=dbg["dbg_kr0"], in_=k_r[0][:])
                nc.sync.dma_start(out=dbg["dbg_va0"], in_=vaug[0][:])

            # ---- attention qb, with wo(tn0) interleaved into qb1
            def attn_S_kb(t, qb, kb):
                """S + exp + mask for one key block: both heads' S blocks live
                in a single 2-bank psum tile so exp is one ACT op per kb."""
                d = kb * 128 - qb * 512
                sd = max(d, 0)
                s2 = ps2.tile([128, 1024], F32, tag="s2", name="s2")
                for hh in range(2):
                    nc.tensor.matmul(
                        out=s2[:, hh * 512 + sd:hh * 512 + 512],
                        lhsT=k_r[t][64 * hh:64 * hh + 64, kb * 128:(kb + 1) * 128],
                        rhs=q_r[t][64 * hh:64 * hh + 64,
                                   qb * 512 + sd:(qb + 1) * 512],
                        start=True, stop=True,
                    )
                p_t = late_small.tile([128, 1024], BF16, tag="p", bufs=28)
                nc.scalar.activation(
                    p_t[:].rearrange("p (g n) -> p g n", g=2)[:, :, sd:512],
                    s2[:].rearrange("p (g n) -> p g n", g=2)[:, :, sd:512],
                    mybir.ActivationFunctionType.Exp,
                    bias=biask[:, kb:kb + 1], scale=0.125)
                if kb >= 4 * qb:    # diagonal block: causal mask
                    s0 = 384 - d
                    for hh in range(2):
                        meng = nc.vector
                        meng.tensor_mul(
                            p_t[:, hh * 512 + sd:hh * 512 + 512],
                            p_t[:, hh * 512 + sd:hh * 512 + 512],
                            tri[:, s0 + sd:s0 + 512])
                return p_t

            def attn_S(t, qb):
                return [attn_S_kb(t, qb, kb) for kb in range(4 * qb + 4)]

            def attn_AV(t, qb, p_ts):
                nkb = 4 * qb + 4
                av = [ps.tile([65, 512], F32, name="av", tag="ps") for _ in range(2)]
                for kb in range(nkb):
                    d = kb * 128 - qb * 512
                    sd = max(d, 0)
                    for hh in range(2):
                        h = 2 * t + hh
                        nc.tensor.matmul(
                            out=av[hh][:, sd:512],
                            lhsT=vaug[kb][:, h * 65:h * 65 + 65],
                            rhs=p_ts[kb][:, hh * 512 + sd:hh * 512 + 512],
                            start=(kb == 0), stop=(kb == nkb - 1),
                        )
                for hh in range(2):
                    rec = small.tile([1, 512], F32, tag="rec", bufs=4)
                    nc.vector.reciprocal(rec, av[hh][64:65, :])
                    bc = small.tile([64, 512], F32, tag="bc", bufs=4)
                    nc.gpsimd.partition_broadcast(bc[:], rec[:], channels=64)
                    nc.vector.tensor_mul(
                        attnT[t][64 * hh:64 * hh + 64, qb * 512:(qb + 1) * 512],
                        av[hh][0:64, :], bc)

            def wo_chain(dm, tn, qi):
                po = ps.tile([128, 512], F32, tag="ps", name="po")
                for kd in range(4):
                    nc.tensor.matmul(
                        out=po[:],
                        lhsT=wov[:, dm, kd],
                        rhs=attnT[kd][:, tn * 512:(tn + 1) * 512],
                        start=(kd == 0), stop=(kd == 3),
                    )
                osb = late_small.tile([128, 512], BF16, tag="osb", bufs=12)
                if tn == 0:
                    nc.vector.tensor_copy(out=osb, in_=po[:])
                    eng = (nc.sync, nc.gpsimd)[qi % 2]
                else:
                    if qi % 2 == 0:
                        nc.scalar.copy(out=osb, in_=po[:])
                    else:
                        nc.vector.tensor_copy(out=osb, in_=po[:])
                    eng = (nc.sync, nc.gpsimd)[qi % 2]
                eng.dma_start(
                    out=outT_d[dm * 128:(dm + 1) * 128, tn * 512:(tn + 1) * 512],
                    in_=osb)

            # qb0: stagger S/AV across t to hide exp latency
            p_q0 = {}
            p_q0[0] = attn_S(0, 0)
            v_chain(4)
            p_q0[1] = attn_S(1, 0)
            v_chain(5)
            p_q0[2] = attn_S(2, 0)
            attn_AV(0, 0, p_q0[0])
            v_chain(6)
            p_q0[3] = attn_S(3, 0)
            attn_AV(1, 0, p_q0[1])
            v_chain(7)
            attn_AV(2, 0, p_q0[2])
            # prefetch qb1 t=0 S pieces around the qb0 tail AVs
            p_q1 = [attn_S_kb(0, 1, 0), attn_S_kb(0, 1, 1), attn_S_kb(0, 1, 2),
                    attn_S_kb(0, 1, 3)]
            attn_AV(3, 0, p_q0[3])

            if debug_taps:
                nc.sync.dma_start(out=dbg["dbg_at0"], in_=attnT[0][:])
            # qb1: wo(tn0) chains interleaved at kb granularity so PE has
            # independent work while ACT catches up on exps
            wo_q = list(range(16))
            qi = 0
            for t in range(4):
                p_ts = p_q1 if t == 0 else []
                for kb in (range(4, 8) if t == 0 else range(8)):
                    p_ts.append(attn_S_kb(t, 1, kb))
                    if kb % 2 == 1 and wo_q:
                        wo_chain(wo_q.pop(0), 0, qi)
                        qi += 1
                attn_AV(t, 1, p_ts)
            while wo_q:
                wo_chain(wo_q.pop(0), 0, qi)
                qi += 1
            for dm in range(16):
                wo_chain(dm, 1, qi)
                qi += 1
            late_ctx2.__exit__(None, None, None)
            late_ctx.__exit__(None, None, None)

    nc.compile()
    return nc


_CACHE = {}


def _get_program():
    if "nc" not in _CACHE:
        _CACHE["nc"] = build_program()
    return _CACHE["nc"]


def _fp8_split(a):
    f8 = ml_dtypes.float8_e4m3
    hi = a.astype(f8)
    lo = (a - hi.astype(np.float32)).astype(f8)
    return hi, lo


def make_in_maps(hidden_states, sequence_mask, Wq, Wkr, Wdk, Wuk, Wuv, Wo):
    hidden_states = np.asarray(hidden_states, dtype=np.float32)
    sequence_mask = np.asarray(sequence_mask).astype(bool)
    Wq, Wkr, Wdk = (np.asarray(a, np.float32) for a in (Wq, Wkr, Wdk))
    Wuk, Wuv, Wo = (np.asarray(a, np.float32) for a in (Wuk, Wuv, Wo))
    bf = ml_dtypes.bfloat16

    inv_freq = (1.0 / (THETA ** (np.arange(0, 32, dtype=np.float32) / 32.0))).astype(np.float32)
    tri = (np.arange(896)[None, :] >= (np.arange(128)[:, None] + 384)).astype(bf)
    # signed rope rotation permutation: dst d<32 <- -src[d+32]; d>=32 <- +src[d-32]
    perm = np.zeros((128, 128), np.float32)
    for hh in range(2):
        for dd in range(32):
            perm[hh * 64 + dd + 32, hh * 64 + dd] = -1.0
            perm[hh * 64 + dd, hh * 64 + dd + 32] = 1.0
    perm = perm.astype(bf)
    pkr = np.zeros((128, 512), np.float32)
    for t in range(4):
        for e in range(2):
            for p in range(16):
                pkr[t * 32 + e * 16 + p, t * 128 + e * 64 + p] = 1.0
    pkr = pkr.astype(bf)

    per_g = []
    for g in range(4):
        wcat = np.concatenate(
            [Wq[:, g * 512:(g + 1) * 512], Wkr[:, g * 128:(g + 1) * 128], Wdk],
            axis=1) * WS  # [2048, 1152]
        wh, wl = _fp8_split(wcat)
        # [2048=(j,i,p), 1152=(fm,m)] -> [128p, j, hl, fm, i, m]
        def packw(a):
            return a.reshape(NJ, 2, 128, NFM, 128).transpose(2, 0, 3, 1, 4)
        wc = np.stack([packw(wh), packw(wl)], axis=2)  # [128, j, hl, fm, i, m]
        wc = np.ascontiguousarray(wc.reshape(128, -1))

        wuk_g = Wuk[:, g * 384:(g + 1) * 384]
        wuk_pad = np.zeros((LR, 4 * 128), np.float32)
        for t in range(4):
            wuk_pad[:, t * 128 + 16:t * 128 + 64] = wuk_g[:, (2 * t) * 48:(2 * t + 1) * 48]
            wuk_pad[:, t * 128 + 80:t * 128 + 128] = wuk_g[:, (2 * t + 1) * 48:(2 * t + 2) * 48]
        # [512=(kd,p), 512=(t,m)] -> [128p, t, kd, m]
        wuk_sb = np.ascontiguousarray(
            wuk_pad.reshape(4, 128, 4, 128).transpose(1, 2, 0, 3).reshape(128, -1)).astype(bf)
        wuv_sb = np.ascontiguousarray(
            Wuv[:, g * 512:(g + 1) * 512].reshape(4, 128, 512).transpose(1, 0, 2)
            .reshape(128, -1)).astype(bf)
        wo_g = Wo[g * 512:(g + 1) * 512, :]  # [512=(kd,p), 2048=(dm,m)]
        wo_sb = np.ascontiguousarray(
            wo_g.reshape(4, 128, 16, 128).transpose(1, 2, 0, 3).reshape(128, -1)).astype(bf)
        per_g.append((wc, wuk_sb, wuv_sb, wo_sb))

    per_b = []
    for b in range(B):
        xs = hidden_states[:, b, :].T * XS  # [2048, 1024]
        xh, xl = _fp8_split(xs)
        # [2048=(j,i,p), 1024] -> [128p, j, hl, i, n]
        def packx(a):
            return a.reshape(NJ, 2, 128, TOK).transpose(2, 0, 1, 3)
        xp = np.stack([packx(xh), packx(xl)], axis=2)  # [128, j, hl, i, n]
        xp = np.ascontiguousarray(xp.reshape(128, -1))
        pos = np.cumsum(sequence_mask[b].astype(np.int32)) - 1
        ang = pos.astype(np.float32)[None, :] * inv_freq[:, None]  # [32, 1024]
        cosP = np.ascontiguousarray(np.tile(np.cos(ang), (4, 1))).astype(np.float32)
        sinP = np.ascontiguousarray(np.tile(np.sin(ang), (4, 1))).astype(np.float32)
        biask = np.ascontiguousarray(
            np.where(sequence_mask[b], 0.0, -30.0).astype(np.float32).reshape(8, 128).T)
        per_b.append((xp, cosP.astype(bf), sinP.astype(bf), biask))

    in_maps = []
    for c in range(8):
        b, g = c // 4, c % 4
        wc, wuk_sb, wuv_sb, wo_sb = per_g[g]
        xp, cosP, sinP, biask = per_b[b]
        in_maps.append({
            "x": xp, "wc": wc, "wuk": wuk_sb, "wuv": wuv_sb, "wo": wo_sb,
            "perm": perm, "pkr": pkr, "cosP": cosP, "sinP": sinP,
            "biask": biask, "tri": tri,
        })
    return in_maps


def kernel(hidden_states, sequence_mask, Wq, Wkr, Wdk, Wuk, Wuv, Wo, _trace=False):
    nc = _get_program()
    in_maps = make_in_maps(hidden_states, sequence_mask, Wq, Wkr, Wdk, Wuk, Wuv, Wo)
    if _trace:
        try:
            res = run_bass_kernel_spmd(nc, in_maps, core_ids=list(range(8)), trace=True)
        except Exception:
            res = run_bass_kernel_spmd(nc, in_maps, core_ids=list(range(8)))
    else:
        res = run_bass_kernel_spmd(nc, in_maps, core_ids=list(range(8)))
    mask = np.asarray(sequence_mask).astype(np.float32)  # [B, S]
    out = np.empty((B, S, D), dtype=np.float32)
    for b in range(B):
        acc = np.zeros((D, TOK), dtype=np.float64)
        for g in range(4):
            acc += res.results[4 * b + g]["outT"].astype(np.float64)
        out[b] = acc.T.astype(np.float32) * mask[b][:, None]
    if _trace:
        kernel._last_results = res
    return out
